# revision 28
# baseline (speedup 1.0000x reference)
"""Trainium2 Bass kernel for nn_CBAMSLayer: spatial-attention CBAM block.

Reference computation (per full input x [32, 256, 56, 56]):
    y  = stack([max_c(x), mean_c(x)])          # [N, 2, H, W]
    y  = conv5x5(y, conv_w)                    # [N, 1, H, W], SAME pad
    y  = batchnorm_train(y, gamma, beta)       # stats over (N, H, W)
    out = x * sigmoid(sigmoid(y))

Sharding: data-parallel over batch, 4 images per core on 8 cores; BN batch
statistics (sum, sumsq of y) are all-reduced across cores.

Per-core layout strategy (all engine ops at partition base 0):
  - x is pre-rounded to fp32r (11-bit mantissa) on the host and kept
    resident in SBUF as 8 tiles [128, 3136] (c-half x image).
  - Channel max: DVE folds the two c-halves (bf16 out), GpSimd reduces
    across partitions (axis=C) into a p-major row, a small scatter DMA
    drops it into "partition space" Cmx[112,(n,b)] where hw = b*112 + p.
  - Channel sum: fp32r ones^T @ x chunks stream through the PE at
    1 cy/row; ScalarE permutes PSUM rows into a p-major SBUF row;
    scatter DMA into Csm partition space.
  - The 5x5 conv becomes 6 accumulated matmuls with host-precomputed
    112x112 matrices (3 row-pair shifts x 2 channels): bf16 for the max
    channel, f32 for the sum channel.
  - BN stats: ScalarE accum_out + 112->1 matmul fold; AllReduce [1,2];
    stats broadcast to partitions via DMA; rstd via DVE Newton rsqrt so
    the ACT engine keeps its sigmoid table loaded the whole kernel.
  - Gate: double sigmoid on ScalarE (narrow, bf16 out), one PE transpose
    to row form, flatten DMA, then replication across 128 partitions
    with K=1 bf16 matmuls; DVE multiplies x tiles into output chunks
    that are DMA'd out as they complete.
  - Small DMAs ride on per-engine queues (scalar/gpsimd/vector) so they
    never queue behind the 1.6 MB x loads or output stores.
"""
import numpy as np

NCORES = 8
NIMG = 4
C = 256
HW = 3136
NB = 28          # hw blocks per image
BW = 112         # block width (2 rows of 56)
EPS = 1e-5
TOTAL_COUNT = NCORES * NIMG * HW

_cache = {}


def _make_wmats(conv_w):
    """GEMM matrices [p_in, 3*112] for y += W^T @ C[:, :, b+db], per channel."""
    wk = np.asarray(conv_w, np.float64).reshape(2, 5, 5).copy()
    wk[1] /= C  # fold mean = sum/C into the weights of the mean channel
    Wm = np.zeros((2, 3, 112, 112), np.float64)
    for h2 in (0, 1):
        for c in range(56):
            for sr in (-2, -1, 0, 1, 2):
                h2p = (h2 + sr) % 2
                db = (h2 + sr - h2p) // 2
                for sc in (-2, -1, 0, 1, 2):
                    cp = c + sc
                    if 0 <= cp < 56:
                        for ch in range(2):
                            Wm[ch, db + 1, h2p * 56 + cp, h2 * 56 + c] += wk[ch, sr + 2, sc + 2]
    # per channel: layout [p_in, i*112 + p_out], i = db+1
    import ml_dtypes
    wmax = np.ascontiguousarray(
        Wm[0].transpose(1, 0, 2).reshape(112, 336)).astype(ml_dtypes.bfloat16)
    wsum = np.ascontiguousarray(
        Wm[1].transpose(1, 0, 2).reshape(112, 336)).astype(np.float32)
    return wmax, wsum


def _build(gamma, beta):
    import concourse.bass as bass
    import concourse.bacc as bacc
    import concourse.tile as tile
    from concourse import mybir, masks
    from contextlib import ExitStack

    F32 = mybir.dt.float32
    F32R = mybir.dt.float32r
    BF16 = mybir.dt.bfloat16
    U32 = mybir.dt.uint32
    AX = mybir.AxisListType
    OP = mybir.AluOpType
    ACT = mybir.ActivationFunctionType

    nc = bacc.Bacc("TRN2", target_bir_lowering=False, debug=False, num_devices=NCORES)
    x = nc.dram_tensor("x", [NIMG, C, HW], F32R, kind="ExternalInput").ap()
    wmx = nc.dram_tensor("wmax", [112, 336], BF16, kind="ExternalInput").ap()
    wsm = nc.dram_tensor("wsum", [112, 336], F32, kind="ExternalInput").ap()
    onesd = nc.dram_tensor("ones", [128, 1], F32R, kind="ExternalInput").ap()
    out = nc.dram_tensor("out", [NIMG, C, HW], F32, kind="ExternalOutput").ap()
    cc_in = nc.dram_tensor("cc_in", [1, 16], F32).ap()
    cc_out = nc.dram_tensor("cc_out", [1, 16], F32, addr_space="Shared").ap()

    with tile.TileContext(nc) as tc, ExitStack() as ctx:
        sb = ctx.enter_context(tc.tile_pool(name="sb", bufs=1))
        mp = ctx.enter_context(tc.tile_pool(name="mp", bufs=3))
        srp = ctx.enter_context(tc.tile_pool(name="srp", bufs=2))
        sfp = ctx.enter_context(tc.tile_pool(name="sfp", bufs=2))
        op_ = ctx.enter_context(tc.tile_pool(name="op", bufs=4))

        # small parameter loads first so they never sit behind the x loads
        Wmx = sb.tile([112, 336], BF16)
        nc.sync.dma_start(out=Wmx[:], in_=wmx)
        Wsm = sb.tile([112, 336], F32)
        nc.sync.dma_start(out=Wsm[:], in_=wsm)
        ones128 = sb.tile([128, 1], F32R)
        nc.sync.dma_start(out=ones128[:], in_=onesd)

        # x loads spread across engine DMA queues
        X = [[sb.tile([128, HW], F32, tag=f"x{n}h{h}", name=f"x{n}h{h}") for h in range(2)]
             for n in range(NIMG)]
        for n in range(NIMG):
            for h in range(2):
                nc.sync.dma_start(
                    out=X[n][h][:].bitcast(F32R), in_=x[n, h * 128:(h + 1) * 128, :])

        identb = sb.tile([128, 128], BF16)
        masks.make_identity(nc, identb[:])
        identf = sb.tile([112, 112], F32)
        masks.make_identity(nc, identf[:])

        Cmx = sb.tile([112, NIMG, 30], BF16)
        Csm = sb.tile([112, NIMG, 30], F32)
        nc.gpsimd.memset(Cmx[:], 0.0)
        nc.gpsimd.memset(Csm[:], 0.0)
        scol = sb.tile([112, 2], F32)
        ysb = sb.tile([112, NIMG, NB], F32)
        strash2 = sb.tile([112, 112], F32)
        ysbT = sb.tile([112, 112], F32)
        s1T = sb.tile([112, 112], F32)
        s2T = sb.tile([112, 112], BF16)
        ones112 = sb.tile([112, 1], F32)
        ocol = sb.tile([1, 128], BF16)
        nc.vector.memset(ones112[:], 1.0)
        nc.vector.memset(ocol[:], 1.0)
        stats_bc = sb.tile([112, 2], F32)
        mean_t = sb.tile([112, 1], F32)
        ve_t = sb.tile([112, 1], F32)
        vh_t = sb.tile([112, 1], F32)
        m2_t = sb.tile([112, 1], F32)
        r_t = sb.tile([112, 1], F32)
        t_t = sb.tile([112, 1], F32)
        scale_t = sb.tile([112, 1], F32)
        bias_t = sb.tile([112, 1], F32)
        st_sb = sb.tile([1, 16], F32)

        with ExitStack() as p2:
            tp = p2.enter_context(tc.tile_pool(name="tp", bufs=3, space="PSUM"))
            sp = p2.enter_context(tc.tile_pool(name="sp", bufs=2, space="PSUM"))
            ytp = p2.enter_context(tc.tile_pool(name="ytp", bufs=1, space="PSUM"))
            pyp = p2.enter_context(tc.tile_pool(name="pyp", bufs=1, space="PSUM"))
            pfp = p2.enter_context(tc.tile_pool(name="pfp", bufs=1, space="PSUM"))

            for n in range(NIMG):
                # ---- channel sum: float32r ones^T @ x chunks (1 cy/row),
                # ScalarE permute to p-major row, scatter-DMA to partition
                # space ----
                srow = srp.tile([1, HW], F32, tag="sr", name="srow")
                for k in range(7):
                    sp_t = sp.tile([1, 448], F32, tag="sp", name="sp_t")
                    for h in range(2):
                        nc.tensor.matmul(
                            sp_t[:],
                            ones128[:],
                            X[n][h][:, k * 448:(k + 1) * 448].bitcast(F32R),
                            start=(h == 0), stop=(h == 1),
                            skip_group_check=True)
                    # srow[p*28 + 4k + b] = sp_t[b*112 + p]
                    nc.scalar.copy(
                        srow.rearrange("q (p k b) -> q k b p", k=7, b=4)[:, k],
                        sp_t[:])
                nc.scalar.dma_start(
                    out=Csm[:, n, 1:29],
                    in_=srow.rearrange("q (p b) -> q p b", b=28))

                # ---- channel max: DVE fold (bf16), PE transpose 7-block
                # groups (1 cy/row bf16), DVE reduce-max to partition
                # space ----
                for c in range(4):
                    lo, hi = c * 784, (c + 1) * 784
                    M = mp.tile([128, 784], BF16, tag="m", name="M")
                    nc.vector.tensor_tensor(out=M[:], in0=X[n][0][:, lo:hi],
                                            in1=X[n][1][:, lo:hi], op=OP.max)
                    pt = tp.tile([112, 7, 128], BF16, tag="tp", name="pt")
                    for t in range(7):
                        nc.tensor.matmul(
                            pt[:, t, :],
                            M[:, t * BW:(t + 1) * BW],
                            identb[:],
                            is_transpose=True,
                            start=True, stop=True,
                            skip_group_check=True,
                        )
                    nc.vector.tensor_reduce(
                        out=Cmx[:, n, 1 + 7 * c:8 + 7 * c], in_=pt[:],
                        axis=AX.X, op=OP.max)

            # ---- conv as 6 accumulated matmuls ----
            yp = pyp.tile([112, NIMG, NB], F32)
            for i, db in enumerate((-1, 0, 1)):
                nc.tensor.matmul(
                    yp[:], Wmx[:, i * 112:(i + 1) * 112],
                    Cmx[:, :, 1 + db:29 + db],
                    start=(i == 0), stop=False,
                    skip_group_check=True)
            for i, db in enumerate((-1, 0, 1)):
                nc.tensor.matmul(
                    yp[:], Wsm[:, i * 112:(i + 1) * 112],
                    Csm[:, :, 1 + db:29 + db],
                    start=False, stop=(i == 2),
                    skip_group_check=True)

            # ---- BN stats; transpose y to row form pre-AR (sigmoid
            # commutes with the transpose since BN scale/bias are global) ----
            nc.scalar.activation(out=ysb[:], in_=yp[:], func=ACT.Copy,
                                 accum_out=scol[:, 0:1])
            yT = ytp.tile([112, 112], F32)
            nc.tensor.matmul(yT[:], ysb.rearrange("p n b -> p (n b)"),
                             identf[:], is_transpose=True,
                             start=True, stop=True, skip_group_check=True)
            nc.scalar.activation(out=strash2[:], in_=yT[:],
                                 func=ACT.Square, accum_out=scol[:, 1:2])
            nc.scalar.copy(ysbT[:], yT[:])
            pf = pfp.tile([1, 2], F32)
            nc.tensor.matmul(pf[0:1, :], ones112[:], scol[:], start=True, stop=True)
            nc.vector.memset(st_sb[:], 0.0)
            nc.scalar.copy(st_sb[:, 0:2], pf[0:1, :])
            nc.gpsimd.dma_start(out=cc_in, in_=st_sb[:])
            nc.gpsimd.collective_compute(
                "AllReduce", OP.add,
                replica_groups=[list(range(NCORES))],
                ins=[cc_in], outs=[cc_out])
            bcast = bass.AP(tensor=cc_out.tensor, offset=cc_out.offset,
                            ap=[[0, 112], [1, 2]])
            nc.sync.dma_start(out=stats_bc[:], in_=bcast)

            # ---- BN scale/bias; rstd = 1/sqrt(var+eps) via Newton on DVE
            # (keeps ScalarE on the sigmoid table set all kernel) ----
            inv = 1.0 / TOTAL_COUNT
            nc.vector.tensor_scalar_mul(mean_t[:], stats_bc[:, 0:1], inv)
            nc.vector.tensor_scalar(out=ve_t[:], in0=stats_bc[:, 1:2],
                                    scalar1=inv, scalar2=EPS,
                                    op0=OP.mult, op1=OP.add)
            nc.vector.tensor_scalar(out=m2_t[:], in0=mean_t[:],
                                    scalar1=mean_t[:], scalar2=-1.0,
                                    op0=OP.mult, op1=OP.mult)
            nc.vector.tensor_tensor(out=ve_t[:], in0=ve_t[:], in1=m2_t[:],
                                    op=OP.add)
            # rstd = 1/sqrt(ve) via bit-trick seed + 2 Newton iterations
            # (u32 add saturates via float path, so C - s = ~(s + ~C))
            nc.vector.tensor_scalar(out=r_t[:].bitcast(U32),
                                    in0=ve_t[:].bitcast(U32),
                                    scalar1=1, scalar2=None,
                                    op0=OP.logical_shift_right)
            nc.vector.tensor_scalar(out=r_t[:].bitcast(U32),
                                    in0=r_t[:].bitcast(U32),
                                    scalar1=0xA0C8A620, scalar2=None,
                                    op0=OP.add)
            nc.vector.tensor_scalar(out=r_t[:].bitcast(U32),
                                    in0=r_t[:].bitcast(U32),
                                    scalar1=0, scalar2=None,
                                    op0=OP.bitwise_not)
            nc.vector.tensor_scalar_mul(vh_t[:], ve_t[:], -0.5)
            for _ in range(3):
                nc.vector.tensor_tensor(out=t_t[:], in0=r_t[:], in1=r_t[:],
                                        op=OP.mult)
                nc.vector.tensor_scalar(out=t_t[:], in0=t_t[:],
                                        scalar1=vh_t[:], scalar2=1.5,
                                        op0=OP.mult, op1=OP.add)
                nc.vector.tensor_tensor(out=r_t[:], in0=r_t[:], in1=t_t[:],
                                        op=OP.mult)
            if float(gamma) != 1.0:
                nc.vector.tensor_scalar_mul(scale_t[:], r_t[:], float(gamma))
            else:
                scale_t = r_t
            nc.vector.tensor_scalar(out=bias_t[:], in0=mean_t[:],
                                    scalar1=scale_t[:], scalar2=-1.0,
                                    op0=OP.mult, op1=OP.mult)
            if float(beta) != 0.0:
                nc.vector.tensor_scalar_add(bias_t[:], bias_t[:], float(beta))

            # ---- gate: sigmoid(sigmoid(scale*y + bias)), in row form ----
            nc.scalar.activation(out=s1T[:], in_=ysbT[:], func=ACT.Sigmoid,
                                 bias=bias_t[:], scale=scale_t[:])
            nc.scalar.activation(out=s2T[:], in_=s1T[:], func=ACT.Sigmoid)

        # ---- gate broadcast + multiply + store ----
        with ExitStack() as p3:
            dp = p3.enter_context(tc.tile_pool(name="dp", bufs=2, space="PSUM"))
            for n in range(NIMG):
                sflat = sfp.tile([1, HW], BF16, tag="sf", name="sflat")
                nc.scalar.dma_start(
                    out=sflat.rearrange("q (p f) -> q p f", p=112),
                    in_=s2T[n * 28:(n + 1) * 28, :])
                ots = [op_.tile([128, HW], F32, tag="ot", name="ot")
                       for _ in range(2)]
                for half in range(2):
                    c0 = half * 1568
                    dt = dp.tile([128, 1568], F32, tag="d", name="dt")
                    for o0, cw in ((0, 512), (512, 512), (1024, 512), (1536, 32)):
                        nc.tensor.matmul(
                            dt[:, o0:o0 + cw], ocol[:],
                            sflat[0:1, c0 + o0:c0 + o0 + cw],
                            start=True, stop=True, skip_group_check=True)
                    for h in range(2):
                        nc.vector.tensor_tensor(
                            out=ots[h][:, c0:c0 + 1568],
                            in0=X[n][h][:, c0:c0 + 1568],
                            in1=dt[:], op=OP.mult)
                for h in range(2):
                    nc.sync.dma_start(
                        out=out[n, h * 128:(h + 1) * 128, :], in_=ots[h][:])

    nc.compile()
    return nc


def _get_nc(gamma, beta):
    key = (round(float(gamma), 9), round(float(beta), 9))
    if key not in _cache:
        _cache[key] = _build(float(gamma), float(beta))
    return _cache[key]


def _round_fp32r(a):
    """Round fp32 to fp32r (8-bit exp, 11-bit mantissa), RNE on bit 12."""
    v = np.ascontiguousarray(a, np.float32).view(np.uint32)
    r = (v + (0x7FF + ((v >> 12) & 1))) & np.uint32(0xFFFFF000)
    return r.view(np.float32)


def _make_in_maps(x, conv_w):
    xs = _round_fp32r(np.asarray(x, np.float32)).reshape(NCORES, NIMG, C, HW)
    wmax, wsum = _make_wmats(conv_w)
    ones = np.ones((128, 1), np.float32)
    return [{"x": xs[i], "wmax": wmax, "wsum": wsum, "ones": ones}
            for i in range(NCORES)]


def kernel(x, conv_w, gamma, beta):
    from concourse.bass_utils import run_bass_kernel_spmd

    g = float(np.asarray(gamma).reshape(-1)[0])
    b = float(np.asarray(beta).reshape(-1)[0])

    nc = _get_nc(g, b)
    in_maps = _make_in_maps(x, conv_w)
    res = run_bass_kernel_spmd(nc, in_maps, list(range(NCORES))).results
    o = np.stack([res[i]["out"] for i in range(NCORES)], axis=0)
    return o.reshape(NCORES * NIMG, C, 56, 56)


# revision 34
# speedup vs baseline: 1.0250x; 1.0250x over previous
"""Trainium2 Bass kernel for nn_CBAMSLayer: spatial-attention CBAM block.

Reference computation (per full input x [32, 256, 56, 56]):
    y  = stack([max_c(x), mean_c(x)])          # [N, 2, H, W]
    y  = conv5x5(y, conv_w)                    # [N, 1, H, W], SAME pad
    y  = batchnorm_train(y, gamma, beta)       # stats over (N, H, W)
    out = x * sigmoid(sigmoid(y))

Sharding: data-parallel over batch, 4 images per core on 8 cores; BN batch
statistics (sum, sumsq of y) are all-reduced across cores.

Per-core layout strategy (all engine ops at partition base 0):
  - x is pre-rounded to fp32r (11-bit mantissa) on the host and kept
    resident in SBUF as 8 tiles [128, 3136] (c-half x image).
  - Channel max: DVE folds the two c-halves (bf16 out), GpSimd reduces
    across partitions (axis=C) into a p-major row, a small scatter DMA
    drops it into "partition space" Cmx[112,(n,b)] where hw = b*112 + p.
  - Channel sum: fp32r ones^T @ x chunks stream through the PE at
    1 cy/row; ScalarE permutes PSUM rows into a p-major SBUF row;
    scatter DMA into Csm partition space.
  - The 5x5 conv becomes 6 accumulated matmuls with host-precomputed
    112x112 matrices (3 row-pair shifts x 2 channels): bf16 for the max
    channel, f32 for the sum channel.
  - BN stats: ScalarE accum_out + 112->1 matmul fold; AllReduce [1,2];
    stats broadcast to partitions via DMA; rstd via DVE Newton rsqrt so
    the ACT engine keeps its sigmoid table loaded the whole kernel.
  - Gate: double sigmoid on ScalarE (narrow, bf16 out), one PE transpose
    to row form, flatten DMA, then replication across 128 partitions
    with K=1 bf16 matmuls; DVE multiplies x tiles into output chunks
    that are DMA'd out as they complete.
  - Small DMAs ride on per-engine queues (scalar/gpsimd/vector) so they
    never queue behind the 1.6 MB x loads or output stores.
"""
import numpy as np

NCORES = 8
NIMG = 4
C = 256
HW = 3136
NB = 28          # hw blocks per image
BW = 112         # block width (2 rows of 56)
EPS = 1e-5
TOTAL_COUNT = NCORES * NIMG * HW

_cache = {}


def _make_wmats(conv_w):
    """GEMM matrices [p_in, 3*112] for y += W^T @ C[:, :, b+db], per channel."""
    wk = np.asarray(conv_w, np.float64).reshape(2, 5, 5).copy()
    wk[1] /= C  # fold mean = sum/C into the weights of the mean channel
    Wm = np.zeros((2, 3, 112, 112), np.float64)
    for h2 in (0, 1):
        for c in range(56):
            for sr in (-2, -1, 0, 1, 2):
                h2p = (h2 + sr) % 2
                db = (h2 + sr - h2p) // 2
                for sc in (-2, -1, 0, 1, 2):
                    cp = c + sc
                    if 0 <= cp < 56:
                        for ch in range(2):
                            Wm[ch, db + 1, h2p * 56 + cp, h2 * 56 + c] += wk[ch, sr + 2, sc + 2]
    # per channel: layout [p_in, i*112 + p_out], i = db+1
    import ml_dtypes
    wmax = np.ascontiguousarray(
        Wm[0].transpose(1, 0, 2).reshape(112, 336)).astype(ml_dtypes.bfloat16)
    wsum = np.ascontiguousarray(
        Wm[1].transpose(1, 0, 2).reshape(112, 336)).astype(np.float32)
    return wmax, wsum


def _build(gamma, beta):
    import concourse.bass as bass
    import concourse.bacc as bacc
    import concourse.tile as tile
    from concourse import mybir, masks
    from contextlib import ExitStack

    F32 = mybir.dt.float32
    F32R = mybir.dt.float32r
    BF16 = mybir.dt.bfloat16
    U32 = mybir.dt.uint32
    FP8 = mybir.dt.float8e4
    AX = mybir.AxisListType
    OP = mybir.AluOpType
    ACT = mybir.ActivationFunctionType

    nc = bacc.Bacc("TRN2", target_bir_lowering=False, debug=False, num_devices=NCORES)
    x = nc.dram_tensor("x", [NIMG, C, HW], F32R, kind="ExternalInput").ap()
    wmx = nc.dram_tensor("wmax", [112, 336], BF16, kind="ExternalInput").ap()
    wsm = nc.dram_tensor("wsum", [112, 336], F32, kind="ExternalInput").ap()
    onesd = nc.dram_tensor("ones", [128, 1], F32R, kind="ExternalInput").ap()
    out = nc.dram_tensor("out", [NIMG, C, HW], F32, kind="ExternalOutput").ap()
    cc_in = nc.dram_tensor("cc_in", [1, 16], F32).ap()
    cc_out = nc.dram_tensor("cc_out", [1, 16], F32, addr_space="Shared").ap()

    with tile.TileContext(nc) as tc, ExitStack() as ctx:
        sb = ctx.enter_context(tc.tile_pool(name="sb", bufs=1))
        mp = ctx.enter_context(tc.tile_pool(name="mp", bufs=3))
        srp = ctx.enter_context(tc.tile_pool(name="srp", bufs=2))
        sfp = ctx.enter_context(tc.tile_pool(name="sfp", bufs=1))
        op_ = ctx.enter_context(tc.tile_pool(name="op", bufs=4))

        # small parameter loads first so they never sit behind the x loads
        Wmx = sb.tile([112, 336], BF16)
        nc.sync.dma_start(out=Wmx[:], in_=wmx)
        Wsm = sb.tile([112, 336], F32)
        nc.sync.dma_start(out=Wsm[:], in_=wsm)
        ones128 = sb.tile([128, 1], F32R)
        nc.sync.dma_start(out=ones128[:], in_=onesd)

        # x loads spread across engine DMA queues
        X = [[sb.tile([128, HW], F32, tag=f"x{n}h{h}", name=f"x{n}h{h}") for h in range(2)]
             for n in range(NIMG)]
        for n in range(NIMG):
            for h in range(2):
                nc.sync.dma_start(
                    out=X[n][h][:].bitcast(F32R), in_=x[n, h * 128:(h + 1) * 128, :])

        dumA = sb.tile([1, 1], F32)
        nc.vector.memset(dumA[:], 0.0)
        nc.scalar.activation(out=dumA[:], in_=dumA[:], func=ACT.Sigmoid)

        identf = sb.tile([112, 112], F32)
        masks.make_identity(nc, identf[:])
        identb = sb.tile([128, 128], BF16)
        masks.make_identity(nc, identb[:])

        Cmx = sb.tile([112, NIMG, 30], BF16)
        Csm = sb.tile([112, NIMG, 30], F32)
        nc.gpsimd.memset(Cmx[:], 0.0)
        nc.gpsimd.memset(Csm[:], 0.0)
        scol = sb.tile([112, 2], F32)
        ysb = sb.tile([112, NIMG, NB], F32)
        strash2 = sb.tile([112, 112], F32)
        ysbT = sb.tile([112, 112], F32)
        s1T = sb.tile([112, 112], F32)
        s2T = sb.tile([112, 112], BF16)
        ones112 = sb.tile([112, 1], F32)
        ocol = sb.tile([1, 128], BF16)
        nc.vector.memset(ones112[:], 1.0)
        nc.vector.memset(ocol[:], 1.0)
        orow112 = sb.tile([1, 112], F32)
        nc.vector.memset(orow112[:], 1.0)
        mean_t = sb.tile([112, 1], F32)
        ve_t = sb.tile([112, 1], F32)
        vh_t = sb.tile([112, 1], F32)
        m2_t = sb.tile([112, 1], F32)
        r_t = sb.tile([112, 1], F32)
        t_t = sb.tile([112, 1], F32)
        scale_t = sb.tile([112, 1], F32)
        bias_t = sb.tile([112, 1], F32)
        st_sb = sb.tile([1, 16], F32)

        with ExitStack() as p2:
            tp = p2.enter_context(tc.tile_pool(name="tp", bufs=2, space="PSUM"))
            sp = p2.enter_context(tc.tile_pool(name="sp", bufs=2, space="PSUM"))
            ytp = p2.enter_context(tc.tile_pool(name="ytp", bufs=1, space="PSUM"))
            pyp = p2.enter_context(tc.tile_pool(name="pyp", bufs=1, space="PSUM"))
            pfp = p2.enter_context(tc.tile_pool(name="pfp", bufs=1, space="PSUM"))

            for n in range(NIMG):
                # ---- channel sum: float32r ones^T @ x chunks (1 cy/row),
                # ScalarE permute to p-major row, scatter-DMA to partition
                # space ----
                srow = srp.tile([1, HW], F32, tag="sr", name="srow")
                for k in range(7):
                    sp_t = sp.tile([1, 448], F32, tag="sp", name="sp_t")
                    for h in range(2):
                        nc.tensor.matmul(
                            sp_t[:],
                            ones128[:],
                            X[n][h][:, k * 448:(k + 1) * 448].bitcast(F32R),
                            start=(h == 0), stop=(h == 1),
                            skip_group_check=True)
                    # srow[p*28 + 4k + b] = sp_t[b*112 + p]
                    nc.scalar.copy(
                        srow.rearrange("q (p k b) -> q k b p", k=7, b=4)[:, k],
                        sp_t[:])
                nc.scalar.dma_start(
                    out=Csm[:, n, 1:29],
                    in_=srow.rearrange("q (p b) -> q p b", b=28))

                # ---- channel max: DVE fold (bf16), PE transpose 7-block
                # groups (1 cy/row bf16), DVE reduce-max to partition
                # space ----
                for c in range(4):
                    lo, hi = c * 784, (c + 1) * 784
                    M = mp.tile([128, 784], BF16, tag="m", name="M")
                    nc.vector.tensor_tensor(out=M[:], in0=X[n][0][:, lo:hi],
                                            in1=X[n][1][:, lo:hi], op=OP.max)
                    pt = tp.tile([112, 7, 128], BF16, tag="tp", name="pt")
                    for t in range(7):
                        nc.tensor.matmul(
                            pt[:, t, :],
                            M[:, t * BW:(t + 1) * BW],
                            identb[:],
                            is_transpose=True,
                            start=True, stop=True,
                            skip_group_check=True,
                        )
                    nc.vector.tensor_reduce(
                        out=Cmx[:, n, 1 + 7 * c:8 + 7 * c], in_=pt[:],
                        axis=AX.X, op=OP.max)

            # ---- conv as 6 accumulated matmuls ----
            yp = pyp.tile([112, NIMG, NB], F32)
            for i, db in enumerate((-1, 0, 1)):
                nc.tensor.matmul(
                    yp[:], Wmx[:, i * 112:(i + 1) * 112],
                    Cmx[:, :, 1 + db:29 + db],
                    start=(i == 0), stop=False,
                    skip_group_check=True)
            for i, db in enumerate((-1, 0, 1)):
                nc.tensor.matmul(
                    yp[:], Wsm[:, i * 112:(i + 1) * 112],
                    Csm[:, :, 1 + db:29 + db],
                    start=False, stop=(i == 2),
                    skip_group_check=True)

            # ---- BN stats; transpose y to row form pre-AR (sigmoid
            # commutes with the transpose since BN scale/bias are global) ----
            nc.scalar.activation(out=ysb[:], in_=yp[:], func=ACT.Copy,
                                 accum_out=scol[:, 0:1])
            yT = ytp.tile([112, 112], F32)
            nc.tensor.matmul(yT[:], ysb.rearrange("p n b -> p (n b)"),
                             identf[:], is_transpose=True,
                             start=True, stop=True, skip_group_check=True)
            nc.scalar.activation(out=strash2[:], in_=yT[:],
                                 func=ACT.Square, accum_out=scol[:, 1:2])
            nc.scalar.copy(ysbT[:], yT[:])
            pf = pfp.tile([1, 2], F32)
            nc.tensor.matmul(pf[0:1, :], ones112[:], scol[:], start=True, stop=True)
            nc.vector.memset(st_sb[:], 0.0)
            nc.scalar.copy(st_sb[:, 0:2], pf[0:1, :])
            nc.gpsimd.dma_start(out=cc_in, in_=st_sb[:])
            nc.gpsimd.collective_compute(
                "AllReduce", OP.add,
                replica_groups=[list(range(NCORES))],
                ins=[cc_in], outs=[cc_out])
            st2 = sb.tile([1, 2], F32)
            nc.sync.dma_start(out=st2[:], in_=cc_out[0:1, 0:2])
            stats_ps = ytp.tile([112, 2], F32, tag="stps", name="stats_ps")
            nc.tensor.matmul(stats_ps[:], orow112[:], st2[:],
                             start=True, stop=True, skip_group_check=True)

            # ---- BN scale/bias; rstd = 1/sqrt(var+eps) via Newton on DVE
            # (keeps ScalarE on the sigmoid table set all kernel) ----
            inv = 1.0 / TOTAL_COUNT
            nc.vector.tensor_scalar_mul(mean_t[:], stats_ps[:, 0:1], inv)
            nc.vector.tensor_scalar(out=ve_t[:], in0=stats_ps[:, 1:2],
                                    scalar1=inv, scalar2=EPS,
                                    op0=OP.mult, op1=OP.add)
            nc.vector.tensor_scalar(out=m2_t[:], in0=mean_t[:],
                                    scalar1=mean_t[:], scalar2=-1.0,
                                    op0=OP.mult, op1=OP.mult)
            nc.vector.tensor_tensor(out=ve_t[:], in0=ve_t[:], in1=m2_t[:],
                                    op=OP.add)
            # rstd = 1/sqrt(ve) via bit-trick seed + 2 Newton iterations
            # (u32 add saturates via float path, so C - s = ~(s + ~C))
            nc.vector.tensor_scalar(out=r_t[:].bitcast(U32),
                                    in0=ve_t[:].bitcast(U32),
                                    scalar1=1, scalar2=None,
                                    op0=OP.logical_shift_right)
            nc.vector.tensor_scalar(out=r_t[:].bitcast(U32),
                                    in0=r_t[:].bitcast(U32),
                                    scalar1=0xA0C8A620, scalar2=None,
                                    op0=OP.add)
            nc.vector.tensor_scalar(out=r_t[:].bitcast(U32),
                                    in0=r_t[:].bitcast(U32),
                                    scalar1=0, scalar2=None,
                                    op0=OP.bitwise_not)
            nc.vector.tensor_scalar_mul(vh_t[:], ve_t[:], -0.5)
            for _ in range(3):
                nc.vector.tensor_tensor(out=t_t[:], in0=r_t[:], in1=r_t[:],
                                        op=OP.mult)
                nc.vector.tensor_scalar(out=t_t[:], in0=t_t[:],
                                        scalar1=vh_t[:], scalar2=1.5,
                                        op0=OP.mult, op1=OP.add)
                nc.vector.tensor_tensor(out=r_t[:], in0=r_t[:], in1=t_t[:],
                                        op=OP.mult)
            if float(gamma) != 1.0:
                nc.vector.tensor_scalar_mul(scale_t[:], r_t[:], float(gamma))
            else:
                scale_t = r_t
            nc.vector.tensor_scalar(out=bias_t[:], in0=mean_t[:],
                                    scalar1=scale_t[:], scalar2=-1.0,
                                    op0=OP.mult, op1=OP.mult)
            if float(beta) != 0.0:
                nc.vector.tensor_scalar_add(bias_t[:], bias_t[:], float(beta))

            # ---- gate: sigmoid(sigmoid(scale*y + bias)), in row form ----
            nc.scalar.activation(out=s1T[:], in_=ysbT[:], func=ACT.Sigmoid,
                                 bias=bias_t[:], scale=scale_t[:])
            nc.scalar.activation(out=s2T[:], in_=s1T[:], func=ACT.Sigmoid)

        # ---- gate broadcast + multiply + store ----
        with ExitStack() as p3:
            dp = p3.enter_context(tc.tile_pool(name="dp", bufs=2, space="PSUM"))
            sflat = sfp.tile([1, NIMG, HW], BF16, tag="sf", name="sflat")
            nc.scalar.dma_start(
                out=sflat.rearrange("q n (p f) -> q n p f", p=112),
                in_=s2T[:])
            for n in range(NIMG):
                ots = [op_.tile([128, HW], F32, tag="ot", name="ot")
                       for _ in range(2)]
                for half in range(2):
                    c0 = half * 1568
                    dt = dp.tile([128, 1568], F32, tag="d", name="dt")
                    for o0, cw in ((0, 512), (512, 512), (1024, 512), (1536, 32)):
                        nc.tensor.matmul(
                            dt[:, o0:o0 + cw], ocol[:],
                            sflat[0:1, n, c0 + o0:c0 + o0 + cw],
                            start=True, stop=True, skip_group_check=True)
                    for h in range(2):
                        nc.vector.tensor_tensor(
                            out=ots[h][:, c0:c0 + 1568],
                            in0=X[n][h][:, c0:c0 + 1568],
                            in1=dt[:], op=OP.mult)
                for h in range(2):
                    nc.sync.dma_start(
                        out=out[n, h * 128:(h + 1) * 128, :], in_=ots[h][:])

    nc.compile()
    return nc


def _get_nc(gamma, beta):
    key = (round(float(gamma), 9), round(float(beta), 9))
    if key not in _cache:
        _cache[key] = _build(float(gamma), float(beta))
    return _cache[key]


def _round_fp32r(a):
    """Round fp32 to fp32r (8-bit exp, 11-bit mantissa), RNE on bit 12."""
    v = np.ascontiguousarray(a, np.float32).view(np.uint32)
    r = (v + (0x7FF + ((v >> 12) & 1))) & np.uint32(0xFFFFF000)
    return r.view(np.float32)


def _make_in_maps(x, conv_w):
    xs = _round_fp32r(np.asarray(x, np.float32)).reshape(NCORES, NIMG, C, HW)
    wmax, wsum = _make_wmats(conv_w)
    ones = np.ones((128, 1), np.float32)
    return [{"x": xs[i], "wmax": wmax, "wsum": wsum, "ones": ones}
            for i in range(NCORES)]


def kernel(x, conv_w, gamma, beta):
    from concourse.bass_utils import run_bass_kernel_spmd

    g = float(np.asarray(gamma).reshape(-1)[0])
    b = float(np.asarray(beta).reshape(-1)[0])

    nc = _get_nc(g, b)
    in_maps = _make_in_maps(x, conv_w)
    res = run_bass_kernel_spmd(nc, in_maps, list(range(NCORES))).results
    o = np.stack([res[i]["out"] for i in range(NCORES)], axis=0)
    return o.reshape(NCORES * NIMG, C, 56, 56)


# revision 35
# speedup vs baseline: 1.0773x; 1.0510x over previous
"""Trainium2 Bass kernel for nn_CBAMSLayer: spatial-attention CBAM block.

Reference computation (per full input x [32, 256, 56, 56]):
    y  = stack([max_c(x), mean_c(x)])          # [N, 2, H, W]
    y  = conv5x5(y, conv_w)                    # [N, 1, H, W], SAME pad
    y  = batchnorm_train(y, gamma, beta)       # stats over (N, H, W)
    out = x * sigmoid(sigmoid(y))

Sharding: data-parallel over batch, 4 images per core on 8 cores; BN batch
statistics (sum, sumsq of y) are all-reduced across cores.

Per-core layout strategy (all engine ops at partition base 0):
  - x is pre-rounded to fp32r (11-bit mantissa) on the host and kept
    resident in SBUF as 8 tiles [128, 3136] (c-half x image).
  - Channel max: DVE folds the two c-halves (bf16 out), GpSimd reduces
    across partitions (axis=C) into a p-major row, a small scatter DMA
    drops it into "partition space" Cmx[112,(n,b)] where hw = b*112 + p.
  - Channel sum: fp32r ones^T @ x chunks stream through the PE at
    1 cy/row; ScalarE permutes PSUM rows into a p-major SBUF row;
    scatter DMA into Csm partition space.
  - The 5x5 conv becomes 6 accumulated matmuls with host-precomputed
    112x112 matrices (3 row-pair shifts x 2 channels): bf16 for the max
    channel, f32 for the sum channel.
  - BN stats: ScalarE accum_out + 112->1 matmul fold; AllReduce [1,2];
    stats broadcast to partitions via DMA; rstd via DVE Newton rsqrt so
    the ACT engine keeps its sigmoid table loaded the whole kernel.
  - Gate: double sigmoid on ScalarE (narrow, bf16 out), one PE transpose
    to row form, flatten DMA, then replication across 128 partitions
    with K=1 bf16 matmuls; DVE multiplies x tiles into output chunks
    that are DMA'd out as they complete.
  - Small DMAs ride on per-engine queues (scalar/gpsimd/vector) so they
    never queue behind the 1.6 MB x loads or output stores.
"""
import numpy as np

NCORES = 8
NIMG = 4
C = 256
HW = 3136
NB = 28          # hw blocks per image
BW = 112         # block width (2 rows of 56)
EPS = 1e-5
TOTAL_COUNT = NCORES * NIMG * HW

_cache = {}


def _make_wmats(conv_w):
    """GEMM matrices [p_in, 3*112] for y += W^T @ C[:, :, b+db], per channel."""
    wk = np.asarray(conv_w, np.float64).reshape(2, 5, 5).copy()
    wk[1] /= C  # fold mean = sum/C into the weights of the mean channel
    Wm = np.zeros((2, 3, 112, 112), np.float64)
    for h2 in (0, 1):
        for c in range(56):
            for sr in (-2, -1, 0, 1, 2):
                h2p = (h2 + sr) % 2
                db = (h2 + sr - h2p) // 2
                for sc in (-2, -1, 0, 1, 2):
                    cp = c + sc
                    if 0 <= cp < 56:
                        for ch in range(2):
                            Wm[ch, db + 1, h2p * 56 + cp, h2 * 56 + c] += wk[ch, sr + 2, sc + 2]
    # per channel: layout [p_in, i*112 + p_out], i = db+1
    import ml_dtypes
    wmax = np.ascontiguousarray(
        Wm[0].transpose(1, 0, 2).reshape(112, 336)).astype(ml_dtypes.bfloat16)
    wsum = np.ascontiguousarray(
        Wm[1].transpose(1, 0, 2).reshape(112, 336)).astype(np.float32)
    return wmax, wsum


def _build(gamma, beta):
    import concourse.bass as bass
    import concourse.bacc as bacc
    import concourse.tile as tile
    from concourse import mybir, masks
    from contextlib import ExitStack

    F32 = mybir.dt.float32
    F32R = mybir.dt.float32r
    BF16 = mybir.dt.bfloat16
    U32 = mybir.dt.uint32
    FP8 = mybir.dt.float8e4
    AX = mybir.AxisListType
    OP = mybir.AluOpType
    ACT = mybir.ActivationFunctionType

    nc = bacc.Bacc("TRN2", target_bir_lowering=False, debug=False, num_devices=NCORES)
    x = nc.dram_tensor("x", [NIMG, C, HW], F32R, kind="ExternalInput").ap()
    wmx = nc.dram_tensor("wmax", [112, 336], BF16, kind="ExternalInput").ap()
    wsm = nc.dram_tensor("wsum", [112, 336], F32, kind="ExternalInput").ap()
    onesd = nc.dram_tensor("ones", [128, 1], F32R, kind="ExternalInput").ap()
    out = nc.dram_tensor("out", [NIMG, C, HW], F32, kind="ExternalOutput").ap()
    cc_in = nc.dram_tensor("cc_in", [1, 16], F32).ap()
    cc_out = nc.dram_tensor("cc_out", [1, 16], F32, addr_space="Shared").ap()

    with tile.TileContext(nc) as tc, ExitStack() as ctx:
        sb = ctx.enter_context(tc.tile_pool(name="sb", bufs=1))
        mp = ctx.enter_context(tc.tile_pool(name="mp", bufs=3))
        srp = ctx.enter_context(tc.tile_pool(name="srp", bufs=2))
        sfp = ctx.enter_context(tc.tile_pool(name="sfp", bufs=1))
        op_ = ctx.enter_context(tc.tile_pool(name="op", bufs=4))

        warmw = sb.tile([128, 128], BF16)
        nc.vector.memset(warmw[:], 0.0)
        for _ in range(2):
            nc.tensor.ldweights(warmw[:], is_transpose=False)

        # small parameter loads first so they never sit behind the x loads
        Wmx = sb.tile([112, 336], BF16)
        nc.sync.dma_start(out=Wmx[:], in_=wmx)
        Wsm = sb.tile([112, 336], F32)
        nc.sync.dma_start(out=Wsm[:], in_=wsm)
        ones128 = sb.tile([128, 1], F32R)
        nc.sync.dma_start(out=ones128[:], in_=onesd)

        # x loads spread across engine DMA queues
        X = [[sb.tile([128, HW], F32, tag=f"x{n}h{h}", name=f"x{n}h{h}") for h in range(2)]
             for n in range(NIMG)]
        for n in range(NIMG):
            for h in range(2):
                nc.sync.dma_start(
                    out=X[n][h][:].bitcast(F32R), in_=x[n, h * 128:(h + 1) * 128, :])

        dumA = sb.tile([1, 1], F32)
        nc.vector.memset(dumA[:], 0.0)
        nc.scalar.activation(out=dumA[:], in_=dumA[:], func=ACT.Sigmoid)

        identf = sb.tile([112, 112], F32)
        masks.make_identity(nc, identf[:])
        identb = sb.tile([128, 128], BF16)
        masks.make_identity(nc, identb[:])

        Cmx = sb.tile([112, NIMG, 30], BF16)
        Csm = sb.tile([112, NIMG, 30], F32)
        nc.gpsimd.memset(Cmx[:], 0.0)
        nc.gpsimd.memset(Csm[:], 0.0)
        scol = sb.tile([112, 2], F32)
        ysb = sb.tile([112, NIMG, NB], F32)
        strash2 = sb.tile([112, 112], F32)
        ysbT = sb.tile([112, 112], F32)
        s1T = sb.tile([112, 112], F32)
        s2T = sb.tile([112, 112], BF16)
        ones112 = sb.tile([112, 1], F32)
        ocol = sb.tile([1, 128], BF16)
        nc.vector.memset(ones112[:], 1.0)
        nc.vector.memset(ocol[:], 1.0)
        orow112 = sb.tile([1, 112], F32)
        nc.vector.memset(orow112[:], 1.0)
        mean_t = sb.tile([112, 1], F32)
        ve_t = sb.tile([112, 1], F32)
        vh_t = sb.tile([112, 1], F32)
        m2_t = sb.tile([112, 1], F32)
        r_t = sb.tile([112, 1], F32)
        t_t = sb.tile([112, 1], F32)
        scale_t = sb.tile([112, 1], F32)
        bias_t = sb.tile([112, 1], F32)
        st_sb = sb.tile([1, 16], F32)

        with ExitStack() as p2:
            tp = p2.enter_context(tc.tile_pool(name="tp", bufs=2, space="PSUM"))
            sp = p2.enter_context(tc.tile_pool(name="sp", bufs=2, space="PSUM"))
            ytp = p2.enter_context(tc.tile_pool(name="ytp", bufs=1, space="PSUM"))
            pyp = p2.enter_context(tc.tile_pool(name="pyp", bufs=1, space="PSUM"))
            pfp = p2.enter_context(tc.tile_pool(name="pfp", bufs=1, space="PSUM"))

            for n in range(NIMG):
                # ---- channel sum: float32r ones^T @ x chunks (1 cy/row),
                # ScalarE permute to p-major row, scatter-DMA to partition
                # space ----
                srow = srp.tile([1, HW], F32, tag="sr", name="srow")
                for k in range(7):
                    sp_t = sp.tile([1, 448], F32, tag="sp", name="sp_t")
                    for h in range(2):
                        nc.tensor.matmul(
                            sp_t[:],
                            ones128[:],
                            X[n][h][:, k * 448:(k + 1) * 448].bitcast(F32R),
                            start=(h == 0), stop=(h == 1),
                            skip_group_check=True)
                    # srow[p*28 + 4k + b] = sp_t[b*112 + p]
                    nc.scalar.copy(
                        srow.rearrange("q (p k b) -> q k b p", k=7, b=4)[:, k],
                        sp_t[:])
                nc.scalar.dma_start(
                    out=Csm[:, n, 1:29],
                    in_=srow.rearrange("q (p b) -> q p b", b=28))

                # ---- channel max: DVE fold (bf16), PE transpose 7-block
                # groups (1 cy/row bf16), DVE reduce-max to partition
                # space ----
                for c in range(4):
                    lo, hi = c * 784, (c + 1) * 784
                    M = mp.tile([128, 784], BF16, tag="m", name="M")
                    nc.vector.tensor_tensor(out=M[:], in0=X[n][0][:, lo:hi],
                                            in1=X[n][1][:, lo:hi], op=OP.max)
                    pt = tp.tile([112, 7, 128], BF16, tag="tp", name="pt")
                    for t in range(7):
                        nc.tensor.matmul(
                            pt[:, t, :],
                            M[:, t * BW:(t + 1) * BW],
                            identb[:],
                            is_transpose=True,
                            start=True, stop=True,
                            skip_group_check=True,
                        )
                    nc.vector.tensor_reduce(
                        out=Cmx[:, n, 1 + 7 * c:8 + 7 * c], in_=pt[:],
                        axis=AX.X, op=OP.max)

            # ---- conv as 6 accumulated matmuls ----
            yp = pyp.tile([112, NIMG, NB], F32)
            for i, db in enumerate((-1, 0, 1)):
                nc.tensor.matmul(
                    yp[:], Wmx[:, i * 112:(i + 1) * 112],
                    Cmx[:, :, 1 + db:29 + db],
                    start=(i == 0), stop=False,
                    skip_group_check=True)
            for i, db in enumerate((-1, 0, 1)):
                nc.tensor.matmul(
                    yp[:], Wsm[:, i * 112:(i + 1) * 112],
                    Csm[:, :, 1 + db:29 + db],
                    start=False, stop=(i == 2),
                    skip_group_check=True)

            # ---- BN stats; transpose y to row form pre-AR (sigmoid
            # commutes with the transpose since BN scale/bias are global) ----
            nc.scalar.activation(out=ysb[:], in_=yp[:], func=ACT.Copy,
                                 accum_out=scol[:, 0:1])
            yT = ytp.tile([112, 112], F32)
            nc.tensor.matmul(yT[:], ysb.rearrange("p n b -> p (n b)"),
                             identf[:], is_transpose=True,
                             start=True, stop=True, skip_group_check=True)
            nc.scalar.activation(out=strash2[:], in_=yT[:],
                                 func=ACT.Square, accum_out=scol[:, 1:2])
            nc.scalar.copy(ysbT[:], yT[:])
            pf = pfp.tile([1, 2], F32)
            nc.tensor.matmul(pf[0:1, :], ones112[:], scol[:], start=True, stop=True)
            nc.vector.memset(st_sb[:], 0.0)
            nc.scalar.copy(st_sb[:, 0:2], pf[0:1, :])
            nc.gpsimd.dma_start(out=cc_in, in_=st_sb[:])
            nc.gpsimd.collective_compute(
                "AllReduce", OP.add,
                replica_groups=[list(range(NCORES))],
                ins=[cc_in], outs=[cc_out])
            st2 = sb.tile([1, 2], F32)
            nc.sync.dma_start(out=st2[:], in_=cc_out[0:1, 0:2])
            stats_ps = ytp.tile([112, 2], F32, tag="stps", name="stats_ps")
            nc.tensor.matmul(stats_ps[:], orow112[:], st2[:],
                             start=True, stop=True, skip_group_check=True)

            # keep the PE clocked up while waiting for the gate row
            for _ in range(25):
                nc.tensor.ldweights(warmw[:], is_transpose=False)

            # ---- BN scale/bias; rstd = 1/sqrt(var+eps) via Newton on DVE
            # (keeps ScalarE on the sigmoid table set all kernel) ----
            inv = 1.0 / TOTAL_COUNT
            nc.vector.tensor_scalar_mul(mean_t[:], stats_ps[:, 0:1], inv)
            nc.vector.tensor_scalar(out=ve_t[:], in0=stats_ps[:, 1:2],
                                    scalar1=inv, scalar2=EPS,
                                    op0=OP.mult, op1=OP.add)
            nc.vector.tensor_scalar(out=m2_t[:], in0=mean_t[:],
                                    scalar1=mean_t[:], scalar2=-1.0,
                                    op0=OP.mult, op1=OP.mult)
            nc.vector.tensor_tensor(out=ve_t[:], in0=ve_t[:], in1=m2_t[:],
                                    op=OP.add)
            # rstd = 1/sqrt(ve) via bit-trick seed + 2 Newton iterations
            # (u32 add saturates via float path, so C - s = ~(s + ~C))
            nc.vector.tensor_scalar(out=r_t[:].bitcast(U32),
                                    in0=ve_t[:].bitcast(U32),
                                    scalar1=1, scalar2=None,
                                    op0=OP.logical_shift_right)
            nc.vector.tensor_scalar(out=r_t[:].bitcast(U32),
                                    in0=r_t[:].bitcast(U32),
                                    scalar1=0xA0C8A620, scalar2=None,
                                    op0=OP.add)
            nc.vector.tensor_scalar(out=r_t[:].bitcast(U32),
                                    in0=r_t[:].bitcast(U32),
                                    scalar1=0, scalar2=None,
                                    op0=OP.bitwise_not)
            nc.vector.tensor_scalar_mul(vh_t[:], ve_t[:], -0.5)
            for _ in range(2):
                nc.vector.tensor_tensor(out=t_t[:], in0=r_t[:], in1=r_t[:],
                                        op=OP.mult)
                nc.vector.tensor_scalar(out=t_t[:], in0=t_t[:],
                                        scalar1=vh_t[:], scalar2=1.5,
                                        op0=OP.mult, op1=OP.add)
                nc.vector.tensor_tensor(out=r_t[:], in0=r_t[:], in1=t_t[:],
                                        op=OP.mult)
            if float(gamma) != 1.0:
                nc.vector.tensor_scalar_mul(scale_t[:], r_t[:], float(gamma))
            else:
                scale_t = r_t
            nc.vector.tensor_scalar(out=bias_t[:], in0=mean_t[:],
                                    scalar1=scale_t[:], scalar2=-1.0,
                                    op0=OP.mult, op1=OP.mult)
            if float(beta) != 0.0:
                nc.vector.tensor_scalar_add(bias_t[:], bias_t[:], float(beta))

            # ---- gate: sigmoid(sigmoid(scale*y + bias)), in row form ----
            nc.scalar.activation(out=s1T[:], in_=ysbT[:], func=ACT.Sigmoid,
                                 bias=bias_t[:], scale=scale_t[:])
            nc.scalar.activation(out=s2T[:], in_=s1T[:], func=ACT.Sigmoid)

        # ---- gate broadcast + multiply + store ----
        with ExitStack() as p3:
            dp = p3.enter_context(tc.tile_pool(name="dp", bufs=2, space="PSUM"))
            sflat = sfp.tile([1, NIMG, HW], BF16, tag="sf", name="sflat")
            nc.scalar.dma_start(
                out=sflat.rearrange("q n (p f) -> q n p f", p=112),
                in_=s2T[:])
            for n in range(NIMG):
                ots = [op_.tile([128, HW], F32, tag="ot", name="ot")
                       for _ in range(2)]
                for half in range(2):
                    c0 = half * 1568
                    dt = dp.tile([128, 1568], F32, tag="d", name="dt")
                    for o0, cw in ((0, 512), (512, 512), (1024, 512), (1536, 32)):
                        nc.tensor.matmul(
                            dt[:, o0:o0 + cw], ocol[:],
                            sflat[0:1, n, c0 + o0:c0 + o0 + cw],
                            start=True, stop=True, skip_group_check=True)
                    for h in range(2):
                        nc.vector.tensor_tensor(
                            out=ots[h][:, c0:c0 + 1568],
                            in0=X[n][h][:, c0:c0 + 1568],
                            in1=dt[:], op=OP.mult)
                for h in range(2):
                    nc.sync.dma_start(
                        out=out[n, h * 128:(h + 1) * 128, :], in_=ots[h][:])

    nc.compile()
    return nc


def _get_nc(gamma, beta):
    key = (round(float(gamma), 9), round(float(beta), 9))
    if key not in _cache:
        _cache[key] = _build(float(gamma), float(beta))
    return _cache[key]


def _round_fp32r(a):
    """Round fp32 to fp32r (8-bit exp, 11-bit mantissa), RNE on bit 12."""
    v = np.ascontiguousarray(a, np.float32).view(np.uint32)
    r = (v + (0x7FF + ((v >> 12) & 1))) & np.uint32(0xFFFFF000)
    return r.view(np.float32)


def _make_in_maps(x, conv_w):
    xs = _round_fp32r(np.asarray(x, np.float32)).reshape(NCORES, NIMG, C, HW)
    wmax, wsum = _make_wmats(conv_w)
    ones = np.ones((128, 1), np.float32)
    return [{"x": xs[i], "wmax": wmax, "wsum": wsum, "ones": ones}
            for i in range(NCORES)]


def kernel(x, conv_w, gamma, beta):
    from concourse.bass_utils import run_bass_kernel_spmd

    g = float(np.asarray(gamma).reshape(-1)[0])
    b = float(np.asarray(beta).reshape(-1)[0])

    nc = _get_nc(g, b)
    in_maps = _make_in_maps(x, conv_w)
    res = run_bass_kernel_spmd(nc, in_maps, list(range(NCORES))).results
    o = np.stack([res[i]["out"] for i in range(NCORES)], axis=0)
    return o.reshape(NCORES * NIMG, C, 56, 56)


# revision 36
# speedup vs baseline: 1.1356x; 1.0541x over previous
"""Trainium2 Bass kernel for nn_CBAMSLayer: spatial-attention CBAM block.

Reference computation (per full input x [32, 256, 56, 56]):
    y  = stack([max_c(x), mean_c(x)])          # [N, 2, H, W]
    y  = conv5x5(y, conv_w)                    # [N, 1, H, W], SAME pad
    y  = batchnorm_train(y, gamma, beta)       # stats over (N, H, W)
    out = x * sigmoid(sigmoid(y))

Sharding: data-parallel over batch, 4 images per core on 8 cores; BN batch
statistics (sum, sumsq of y) are all-reduced across cores.

Per-core layout strategy (all engine ops at partition base 0):
  - x is pre-rounded to fp32r (11-bit mantissa) on the host and kept
    resident in SBUF as 8 tiles [128, 3136] (c-half x image).
  - Channel max: DVE folds the two c-halves (bf16 out), GpSimd reduces
    across partitions (axis=C) into a p-major row, a small scatter DMA
    drops it into "partition space" Cmx[112,(n,b)] where hw = b*112 + p.
  - Channel sum: fp32r ones^T @ x chunks stream through the PE at
    1 cy/row; ScalarE permutes PSUM rows into a p-major SBUF row;
    scatter DMA into Csm partition space.
  - The 5x5 conv becomes 6 accumulated matmuls with host-precomputed
    112x112 matrices (3 row-pair shifts x 2 channels): bf16 for the max
    channel, f32 for the sum channel.
  - BN stats: ScalarE accum_out + 112->1 matmul fold; AllReduce [1,2];
    stats broadcast to partitions via DMA; rstd via DVE Newton rsqrt so
    the ACT engine keeps its sigmoid table loaded the whole kernel.
  - Gate: double sigmoid on ScalarE (narrow, bf16 out), one PE transpose
    to row form, flatten DMA, then replication across 128 partitions
    with K=1 bf16 matmuls; DVE multiplies x tiles into output chunks
    that are DMA'd out as they complete.
  - Small DMAs ride on per-engine queues (scalar/gpsimd/vector) so they
    never queue behind the 1.6 MB x loads or output stores.
"""
import numpy as np

NCORES = 8
NIMG = 4
C = 256
HW = 3136
NB = 28          # hw blocks per image
BW = 112         # block width (2 rows of 56)
EPS = 1e-5
TOTAL_COUNT = NCORES * NIMG * HW

_cache = {}


def _make_wmats(conv_w):
    """GEMM matrices [p_in, 3*112] for y += W^T @ C[:, :, b+db], per channel."""
    wk = np.asarray(conv_w, np.float64).reshape(2, 5, 5).copy()
    wk[1] /= C  # fold mean = sum/C into the weights of the mean channel
    Wm = np.zeros((2, 3, 112, 112), np.float64)
    for h2 in (0, 1):
        for c in range(56):
            for sr in (-2, -1, 0, 1, 2):
                h2p = (h2 + sr) % 2
                db = (h2 + sr - h2p) // 2
                for sc in (-2, -1, 0, 1, 2):
                    cp = c + sc
                    if 0 <= cp < 56:
                        for ch in range(2):
                            Wm[ch, db + 1, h2p * 56 + cp, h2 * 56 + c] += wk[ch, sr + 2, sc + 2]
    # per channel: layout [p_in, i*112 + p_out], i = db+1
    import ml_dtypes
    wmax = np.ascontiguousarray(
        Wm[0].transpose(1, 0, 2).reshape(112, 336)).astype(ml_dtypes.bfloat16)
    wsum = np.ascontiguousarray(
        Wm[1].transpose(1, 0, 2).reshape(112, 336)).astype(np.float32)
    return wmax, wsum


def _build(gamma, beta):
    import concourse.bass as bass
    import concourse.bacc as bacc
    import concourse.tile as tile
    from concourse import mybir, masks
    from contextlib import ExitStack

    F32 = mybir.dt.float32
    F32R = mybir.dt.float32r
    BF16 = mybir.dt.bfloat16
    U32 = mybir.dt.uint32
    FP8 = mybir.dt.float8e4
    AX = mybir.AxisListType
    OP = mybir.AluOpType
    ACT = mybir.ActivationFunctionType

    nc = bacc.Bacc("TRN2", target_bir_lowering=False, debug=False, num_devices=NCORES)
    x = nc.dram_tensor("x", [NIMG, C, HW], F32R, kind="ExternalInput").ap()
    wmx = nc.dram_tensor("wmax", [112, 336], BF16, kind="ExternalInput").ap()
    wsm = nc.dram_tensor("wsum", [112, 336], F32, kind="ExternalInput").ap()
    onesd = nc.dram_tensor("ones", [128, 1], F32R, kind="ExternalInput").ap()
    out = nc.dram_tensor("out", [NIMG, C, HW], F32, kind="ExternalOutput").ap()
    cc_in = nc.dram_tensor("cc_in", [1, 16], F32).ap()
    cc_out = nc.dram_tensor("cc_out", [1, 16], F32, addr_space="Shared").ap()
    ccw_in = nc.dram_tensor("ccw_in", [1, 16], F32).ap()
    ccw_out = nc.dram_tensor("ccw_out", [1, 16], F32, addr_space="Shared").ap()

    with tile.TileContext(nc) as tc, ExitStack() as ctx:
        sb = ctx.enter_context(tc.tile_pool(name="sb", bufs=1))
        mp = ctx.enter_context(tc.tile_pool(name="mp", bufs=3))
        srp = ctx.enter_context(tc.tile_pool(name="srp", bufs=2))
        sfp = ctx.enter_context(tc.tile_pool(name="sfp", bufs=1))
        op_ = ctx.enter_context(tc.tile_pool(name="op", bufs=4))

        # warm-up collective: absorbs the entry barrier and aligns the
        # cores long before the real stats AllReduce
        ccw_sb = sb.tile([1, 16], F32)
        nc.vector.memset(ccw_sb[:], 0.0)
        nc.gpsimd.dma_start(out=ccw_in, in_=ccw_sb[:])
        nc.gpsimd.collective_compute(
            "AllReduce", OP.add,
            replica_groups=[list(range(NCORES))],
            ins=[ccw_in], outs=[ccw_out])

        # small parameter loads first so they never sit behind the x loads
        Wmx = sb.tile([112, 336], BF16)
        nc.sync.dma_start(out=Wmx[:], in_=wmx)
        Wsm = sb.tile([112, 336], F32)
        nc.sync.dma_start(out=Wsm[:], in_=wsm)
        ones128 = sb.tile([128, 1], F32R)
        nc.sync.dma_start(out=ones128[:], in_=onesd)

        # x loads spread across engine DMA queues
        X = [[sb.tile([128, HW], F32, tag=f"x{n}h{h}", name=f"x{n}h{h}") for h in range(2)]
             for n in range(NIMG)]
        for n in range(NIMG):
            for h in range(2):
                nc.sync.dma_start(
                    out=X[n][h][:].bitcast(F32R), in_=x[n, h * 128:(h + 1) * 128, :])

        dumA = sb.tile([1, 1], F32)
        nc.vector.memset(dumA[:], 0.0)
        nc.scalar.activation(out=dumA[:], in_=dumA[:], func=ACT.Sigmoid)

        identf = sb.tile([112, 112], F32)
        masks.make_identity(nc, identf[:])
        identb = sb.tile([128, 128], BF16)
        masks.make_identity(nc, identb[:])

        Cmx = sb.tile([112, NIMG, 30], BF16)
        Csm = sb.tile([112, NIMG, 30], F32)
        nc.gpsimd.memset(Cmx[:], 0.0)
        nc.gpsimd.memset(Csm[:], 0.0)
        scol = sb.tile([112, 2], F32)
        ysb = sb.tile([112, NIMG, NB], F32)
        strash2 = sb.tile([112, 112], F32)
        ysbT = sb.tile([112, 112], F32)
        s1T = sb.tile([112, 112], F32)
        s2T = sb.tile([112, 112], BF16)
        ones112 = sb.tile([112, 1], F32)
        ocol = sb.tile([1, 128], BF16)
        nc.vector.memset(ones112[:], 1.0 / TOTAL_COUNT)
        epsv = sb.tile([1, 2], F32)
        nc.vector.memset(epsv[:, 0:1], 0.0)
        nc.vector.memset(epsv[:, 1:2], EPS / NCORES)
        nc.vector.memset(ocol[:], 1.0)
        orow112 = sb.tile([1, 112], F32)
        nc.vector.memset(orow112[:], 1.0)
        ve_t = sb.tile([112, 1], F32)
        vh_t = sb.tile([112, 1], F32)
        m2_t = sb.tile([112, 1], F32)
        r_t = sb.tile([112, 1], F32)
        t_t = sb.tile([112, 1], F32)
        scale_t = sb.tile([112, 1], F32)
        bias_t = sb.tile([112, 1], F32)
        st_sb = sb.tile([1, 16], F32)

        with ExitStack() as p2:
            tp = p2.enter_context(tc.tile_pool(name="tp", bufs=2, space="PSUM"))
            sp = p2.enter_context(tc.tile_pool(name="sp", bufs=2, space="PSUM"))
            ytp = p2.enter_context(tc.tile_pool(name="ytp", bufs=1, space="PSUM"))
            pyp = p2.enter_context(tc.tile_pool(name="pyp", bufs=1, space="PSUM"))
            pfp = p2.enter_context(tc.tile_pool(name="pfp", bufs=1, space="PSUM"))

            for n in range(NIMG):
                # ---- channel sum: float32r ones^T @ x chunks (1 cy/row),
                # ScalarE permute to p-major row, scatter-DMA to partition
                # space ----
                srow = srp.tile([1, HW], F32, tag="sr", name="srow")
                for k in range(7):
                    sp_t = sp.tile([1, 448], F32, tag="sp", name="sp_t")
                    for h in range(2):
                        nc.tensor.matmul(
                            sp_t[:],
                            ones128[:],
                            X[n][h][:, k * 448:(k + 1) * 448].bitcast(F32R),
                            start=(h == 0), stop=(h == 1),
                            skip_group_check=True)
                    # srow[p*28 + 4k + b] = sp_t[b*112 + p]
                    nc.scalar.copy(
                        srow.rearrange("q (p k b) -> q k b p", k=7, b=4)[:, k],
                        sp_t[:])
                nc.scalar.dma_start(
                    out=Csm[:, n, 1:29],
                    in_=srow.rearrange("q (p b) -> q p b", b=28))

                # ---- channel max: DVE fold (bf16), PE transpose 7-block
                # groups (1 cy/row bf16), DVE reduce-max to partition
                # space ----
                for c in range(4):
                    lo, hi = c * 784, (c + 1) * 784
                    M = mp.tile([128, 784], BF16, tag="m", name="M")
                    nc.vector.tensor_tensor(out=M[:], in0=X[n][0][:, lo:hi],
                                            in1=X[n][1][:, lo:hi], op=OP.max)
                    pt = tp.tile([112, 7, 128], BF16, tag="tp", name="pt")
                    for t in range(7):
                        nc.tensor.matmul(
                            pt[:, t, :],
                            M[:, t * BW:(t + 1) * BW],
                            identb[:],
                            is_transpose=True,
                            start=True, stop=True,
                            skip_group_check=True,
                        )
                    nc.vector.tensor_reduce(
                        out=Cmx[:, n, 1 + 7 * c:8 + 7 * c], in_=pt[:],
                        axis=AX.X, op=OP.max)

            # ---- conv as 6 accumulated matmuls ----
            yp = pyp.tile([112, NIMG, NB], F32)
            for i, db in enumerate((-1, 0, 1)):
                nc.tensor.matmul(
                    yp[:], Wmx[:, i * 112:(i + 1) * 112],
                    Cmx[:, :, 1 + db:29 + db],
                    start=(i == 0), stop=False,
                    skip_group_check=True)
            for i, db in enumerate((-1, 0, 1)):
                nc.tensor.matmul(
                    yp[:], Wsm[:, i * 112:(i + 1) * 112],
                    Csm[:, :, 1 + db:29 + db],
                    start=False, stop=(i == 2),
                    skip_group_check=True)

            # ---- BN stats; transpose y to row form pre-AR (sigmoid
            # commutes with the transpose since BN scale/bias are global) ----
            nc.scalar.activation(out=ysb[:], in_=yp[:], func=ACT.Copy,
                                 accum_out=scol[:, 0:1])
            yT = ytp.tile([112, 112], F32)
            nc.tensor.matmul(yT[:], ysb.rearrange("p n b -> p (n b)"),
                             identf[:], is_transpose=True,
                             start=True, stop=True, skip_group_check=True)
            nc.scalar.activation(out=strash2[:], in_=yT[:],
                                 func=ACT.Square, accum_out=scol[:, 1:2])
            nc.scalar.copy(ysbT[:], yT[:])
            pf = pfp.tile([1, 2], F32)
            nc.tensor.matmul(pf[0:1, :], ones112[:], scol[:], start=True, stop=True)
            nc.vector.memset(st_sb[:], 0.0)
            nc.vector.tensor_tensor(out=st_sb[:, 0:2], in0=pf[0:1, :],
                                    in1=epsv[:], op=OP.add)
            nc.gpsimd.dma_start(out=cc_in, in_=st_sb[:])
            nc.gpsimd.collective_compute(
                "AllReduce", OP.add,
                replica_groups=[list(range(NCORES))],
                ins=[cc_in], outs=[cc_out])
            st2 = sb.tile([1, 2], F32)
            nc.sync.dma_start(out=st2[:], in_=cc_out[0:1, 0:2])
            stats_ps = ytp.tile([112, 2], F32, tag="stps", name="stats_ps")
            nc.tensor.matmul(stats_ps[:], orow112[:], st2[:],
                             start=True, stop=True, skip_group_check=True)

            # ---- BN scale/bias; rstd = 1/sqrt(var+eps) via Newton on DVE
            # (keeps ScalarE on the sigmoid table set all kernel) ----
            mean_t = stats_ps[:, 0:1]
            nc.vector.tensor_scalar(out=m2_t[:], in0=mean_t,
                                    scalar1=mean_t, scalar2=-1.0,
                                    op0=OP.mult, op1=OP.mult)
            nc.vector.tensor_tensor(out=ve_t[:], in0=stats_ps[:, 1:2],
                                    in1=m2_t[:], op=OP.add)
            # rstd = 1/sqrt(ve) via bit-trick seed + 2 Newton iterations
            # (u32 add saturates via float path, so C - s = ~(s + ~C))
            nc.vector.tensor_scalar(out=r_t[:].bitcast(U32),
                                    in0=ve_t[:].bitcast(U32),
                                    scalar1=1, scalar2=None,
                                    op0=OP.logical_shift_right)
            nc.vector.tensor_scalar(out=r_t[:].bitcast(U32),
                                    in0=r_t[:].bitcast(U32),
                                    scalar1=0xA0C8A620, scalar2=None,
                                    op0=OP.add)
            nc.vector.tensor_scalar(out=r_t[:].bitcast(U32),
                                    in0=r_t[:].bitcast(U32),
                                    scalar1=0, scalar2=None,
                                    op0=OP.bitwise_not)
            nc.vector.tensor_scalar_mul(vh_t[:], ve_t[:], -0.5)
            for _ in range(2):
                nc.vector.tensor_tensor(out=t_t[:], in0=r_t[:], in1=r_t[:],
                                        op=OP.mult)
                nc.vector.tensor_scalar(out=t_t[:], in0=t_t[:],
                                        scalar1=vh_t[:], scalar2=1.5,
                                        op0=OP.mult, op1=OP.add)
                nc.vector.tensor_tensor(out=r_t[:], in0=r_t[:], in1=t_t[:],
                                        op=OP.mult)
            if float(gamma) != 1.0:
                nc.vector.tensor_scalar_mul(scale_t[:], r_t[:], float(gamma))
            else:
                scale_t = r_t
            nc.vector.tensor_scalar(out=bias_t[:], in0=mean_t,
                                    scalar1=scale_t[:], scalar2=-1.0,
                                    op0=OP.mult, op1=OP.mult)
            if float(beta) != 0.0:
                nc.vector.tensor_scalar_add(bias_t[:], bias_t[:], float(beta))

            # ---- gate: sigmoid(sigmoid(scale*y + bias)), in row form ----
            nc.scalar.activation(out=s1T[:], in_=ysbT[:], func=ACT.Sigmoid,
                                 bias=bias_t[:], scale=scale_t[:])
            nc.scalar.activation(out=s2T[:], in_=s1T[:], func=ACT.Sigmoid)

        # ---- gate broadcast + multiply + store ----
        with ExitStack() as p3:
            dp = p3.enter_context(tc.tile_pool(name="dp", bufs=2, space="PSUM"))
            sflat = sfp.tile([1, NIMG, HW], BF16, tag="sf", name="sflat")
            nc.scalar.dma_start(
                out=sflat.rearrange("q n (p f) -> q n p f", p=112),
                in_=s2T[:])
            for n in range(NIMG):
                ots = [op_.tile([128, HW], F32, tag="ot", name="ot")
                       for _ in range(2)]
                for half in range(2):
                    c0 = half * 1568
                    dt = dp.tile([128, 1568], F32, tag="d", name="dt")
                    for o0, cw in ((0, 512), (512, 512), (1024, 512), (1536, 32)):
                        nc.tensor.matmul(
                            dt[:, o0:o0 + cw], ocol[:],
                            sflat[0:1, n, c0 + o0:c0 + o0 + cw],
                            start=True, stop=True, skip_group_check=True)
                    for h in range(2):
                        nc.vector.tensor_tensor(
                            out=ots[h][:, c0:c0 + 1568],
                            in0=X[n][h][:, c0:c0 + 1568],
                            in1=dt[:], op=OP.mult)
                for h in range(2):
                    nc.sync.dma_start(
                        out=out[n, h * 128:(h + 1) * 128, :], in_=ots[h][:])

    nc.compile()
    return nc


def _get_nc(gamma, beta):
    key = (round(float(gamma), 9), round(float(beta), 9))
    if key not in _cache:
        _cache[key] = _build(float(gamma), float(beta))
    return _cache[key]


def _round_fp32r(a):
    """Round fp32 to fp32r (8-bit exp, 11-bit mantissa), RNE on bit 12."""
    v = np.ascontiguousarray(a, np.float32).view(np.uint32)
    r = (v + (0x7FF + ((v >> 12) & 1))) & np.uint32(0xFFFFF000)
    return r.view(np.float32)


def _make_in_maps(x, conv_w):
    xs = _round_fp32r(np.asarray(x, np.float32)).reshape(NCORES, NIMG, C, HW)
    wmax, wsum = _make_wmats(conv_w)
    ones = np.ones((128, 1), np.float32)
    return [{"x": xs[i], "wmax": wmax, "wsum": wsum, "ones": ones}
            for i in range(NCORES)]


def kernel(x, conv_w, gamma, beta):
    from concourse.bass_utils import run_bass_kernel_spmd

    g = float(np.asarray(gamma).reshape(-1)[0])
    b = float(np.asarray(beta).reshape(-1)[0])

    nc = _get_nc(g, b)
    in_maps = _make_in_maps(x, conv_w)
    res = run_bass_kernel_spmd(nc, in_maps, list(range(NCORES))).results
    o = np.stack([res[i]["out"] for i in range(NCORES)], axis=0)
    return o.reshape(NCORES * NIMG, C, 56, 56)
